# revision 3
# baseline (speedup 1.0000x reference)
# CGAT Trainium2 kernel, v5: bf16 aggregation, fused-accum scores, Prelu.
#
# Reference (B=2, V=8192, T=8, F0=F1=32):
#   h  = lrelu(einsum('bvtf,gf->bvtg', x, W_w) + W_b)
#   ht = mean_b(einsum('bvtg,t->bvg', h, Wt_w))
#   s_src = ht @ a_w[:32] ; s_dst = ht @ a_w[32:]
#   e = lrelu(s_dst[:,None] + s_src[None,:]) ; mask adj>0 ; softmax_j
#   out = lrelu(attn @ h)
#
# Distribution: destination rows sharded 8 ways, inputs rolled per core
# (own rows at block 0), no collectives.
#
# Score algebra: w_ij = exp(lrelu(sd_i+ss_j)) = E1_i * e1_j * max(G_i g_j, 1)
# with E1_i dropped (softmax row-invariance) and e1_j folded into the
# u-op via max(a*c, c) = c*max(a, 1):
#   u_ij = max(grep_i * a_j, e2_j)     (ONE dual-AP-scalar tensor_scalar)
#     a_j    = e^{0.2 ss_j} ; e2_j = e^{ss_j} ; grep_i = e^{-0.8 sd_i}
#   p = u * adj   (tensor_tensor, bf16 2x mode, 2 j-blocks per op)
# Aggregation p-stationary bf16: out_ps[ib] += p.T @ h ; z_ps += p.T @ 1.
# (fp8/DoubleRow was measured 2x faster on PE but e4m3/e5m2 quantization
# alone costs 3.2%/3.3% max-rel error -- over the 2e-2 budget.)
#
# s_src/s_dst via scalar_tensor_tensor with accum_out directly on the h
# tiles (one fused multiply+row-sum per v-block, lands in [128, NVB]
# layout -- no PE matmuls, no DRAM transpose).  lrelu of h via ACT
# Prelu (the parametric_relu table honours alpha; Lrelu does NOT).

import os

import numpy as np
import ml_dtypes

B, V, T, F0, F1 = 2, 8192, 8, 32, 32
NCORES = 8
SHARD = V // NCORES
NVB = V // 128               # 64 j-blocks
BTG = B * T * F1             # 512 feature columns
NPASS = 2
IHALF = SHARD // NPASS       # 512
JGRP = 8                     # adj DMA groups 8 j-blocks
ALPHA = 0.2

_prog_cache = {}


def _build_program(with_bias: bool):
    import concourse.bacc as bacc
    import concourse.mybir as mybir
    import concourse.tile as tile

    nc = bacc.Bacc("TRN2", target_bir_lowering=False, debug=False,
                   num_devices=NCORES)
    f32 = mybir.dt.float32
    bf16 = mybir.dt.bfloat16
    AF = mybir.ActivationFunctionType
    OP = mybir.AluOpType

    x_d = nc.dram_tensor("xt", [B, 2, 4, 128, 2048], bf16,
                         kind="ExternalInput").ap()
    adj_d = nc.dram_tensor("adjt", [NPASS, NVB // JGRP, JGRP, 128, IHALF],
                           bf16, kind="ExternalInput").ap()
    wblk_d = nc.dram_tensor("wblk", [128, 128], bf16, kind="ExternalInput").ap()
    wa1_d = nc.dram_tensor("wa1", [128, BTG], bf16, kind="ExternalInput").ap()
    wa2_d = nc.dram_tensor("wa2", [128, BTG], bf16, kind="ExternalInput").ap()
    out_d = nc.dram_tensor("out", [B, SHARD, T, F1], bf16,
                           kind="ExternalOutput").ap()
    if with_bias:
        wb_d = nc.dram_tensor("wb", [1, BTG], f32, kind="ExternalInput").ap()
    sd_scr = nc.dram_tensor("sd_scr", [SHARD], f32, kind="Internal").ap()

    with tile.TileContext(nc) as tc:
        with tc.tile_pool(name="consts", bufs=1) as consts:
            wblk = consts.tile([128, 128], bf16)
            nc.sync.dma_start(wblk[:], wblk_d)
            wa1 = consts.tile([128, BTG], bf16)
            nc.sync.dma_start(wa1[:], wa1_d)
            wa2 = consts.tile([128, BTG], bf16)
            nc.sync.dma_start(wa2[:], wa2_d)
            if with_bias:
                wb_sb = consts.tile([128, BTG], f32)
                nc.gpsimd.dma_start(wb_sb[:], wb_d.partition_broadcast(128))

            hb_sb = consts.tile([128, NVB * BTG], bf16)  # h, jb-major
            ss_t = consts.tile([128, NVB], f32)          # s_src [p, jb]
            sdcol = consts.tile([128, SHARD // 128], f32)  # s_dst own shard
            acol = consts.tile([128, NVB], f32)          # e^{0.2 ss}
            e2col = consts.tile([128, NVB], f32)         # e^{ss}
            grep = consts.tile([128, SHARD], bf16)       # e^{-0.8 sd}
            onecol = consts.tile([128, 1], bf16)
            nc.vector.memset(onecol[:], 1.0)

            # ---- fused phases: B (h + s) with C-pass-0 one group behind ----
            with (
                tc.tile_pool(name="xb", bufs=8) as xb_pool,
                tc.tile_pool(name="hps", bufs=3, space="PSUM") as hps_pool,
                tc.tile_pool(name="hc", bufs=8) as hc_pool,
                tc.tile_pool(name="adjp", bufs=5) as adj_pool,
                tc.tile_pool(name="up", bufs=6) as u_pool,
                tc.tile_pool(name="pp", bufs=6) as p_pool,
                tc.tile_pool(name="ops", bufs=4, space="PSUM") as out_ps_pool,
                tc.tile_pool(name="zps", bufs=1, space="PSUM") as z_ps_pool,
                tc.tile_pool(name="fin", bufs=4) as fin_pool,
            ):
                npair = NVB // 2
                all_out_ps = {}
                all_z_ps = {}

                def emit_pass_tiles(ip):
                    all_out_ps[ip] = [
                        out_ps_pool.tile([128, BTG], f32, tag="out_ps",
                                         name=f"out_ps_{ip}_{k}")
                        for k in range(4)]
                    all_z_ps[ip] = z_ps_pool.tile([128, 4], f32, tag="z_ps",
                                                  name=f"z_ps_{ip}")

                def emit_jbg(ip, jbg, dma_eng):
                    i0 = ip * IHALF
                    gh = grep[:, i0:i0 + IHALF]
                    out_ps = all_out_ps[ip]
                    z_ps = all_z_ps[ip]
                    adjq = adj_pool.tile([128, JGRP * IHALF], bf16,
                                         tag="adjq", name=f"aq_{ip}_{jbg}")
                    dma_eng.dma_start(
                        adjq[:].rearrange("p (k f) -> p k f", k=JGRP),
                        adj_d[ip, jbg].rearrange("k p f -> p k f"))
                    for jp2 in range(JGRP // 2):
                        pr = jbg * (JGRP // 2) + jp2
                        jb = 2 * pr
                        u = u_pool.tile([128, 2 * IHALF], bf16, tag="u",
                                        name=f"u_{ip}_{pr}")
                        nc.vector.tensor_scalar(
                            out=u[:, :IHALF], in0=gh,
                            scalar1=acol[:, jb:jb + 1], op0=OP.mult,
                            scalar2=e2col[:, jb:jb + 1], op1=OP.max)
                        nc.vector.tensor_scalar(
                            out=u[:, IHALF:], in0=gh,
                            scalar1=acol[:, jb + 1:jb + 2], op0=OP.mult,
                            scalar2=e2col[:, jb + 1:jb + 2], op1=OP.max)
                        pt = p_pool.tile([128, 2 * IHALF], bf16, tag="pt",
                                         name=f"pt_{ip}_{pr}")
                        a0 = jp2 * 2 * IHALF
                        nc.vector.tensor_tensor(
                            pt[:], u[:],
                            adjq[:, a0:a0 + 2 * IHALF], op=OP.mult)
                        first, last = pr == 0, pr == npair - 1
                        for half in range(2):
                            jbh = jb + half
                            rhsh = hb_sb[:, jbh * BTG:(jbh + 1) * BTG]
                            for ib in range(4):
                                lhs = pt[:, half * IHALF + ib * 128:
                                         half * IHALF + (ib + 1) * 128]
                                nc.tensor.matmul(
                                    out_ps[ib][:], lhsT=lhs, rhs=rhsh,
                                    start=first and half == 0,
                                    stop=last and half == 1)
                                nc.tensor.matmul(
                                    z_ps[:, ib:ib + 1], lhsT=lhs,
                                    rhs=onecol[:],
                                    start=(first and half == 0 and ib == 0),
                                    stop=(last and half == 1 and ib == 3))

                def emit_finals(ip):
                    i0 = ip * IHALF
                    out_ps = all_out_ps[ip]
                    z_ps = all_z_ps[ip]
                    rz = fin_pool.tile([128, 4], f32, tag="rz",
                                       name=f"rz_{ip}")
                    nc.vector.reciprocal(rz[:], z_ps[:])
                    for ib in range(4):
                        fin = fin_pool.tile([128, BTG], bf16, tag="fin",
                                            name=f"fin_{ip}_{ib}")
                        nc.scalar.activation(fin[:], out_ps[ib][:], AF.Prelu,
                                             bias=0.0, scale=rz[:, ib:ib + 1],
                                             alpha=ALPHA)
                        r0 = i0 + ib * 128
                        dst = out_d[:, r0:r0 + 128, :, :].rearrange(
                            "b i t g -> i b t g")
                        nc.gpsimd.dma_start(
                            dst,
                            fin[:].rearrange("i (b t g) -> i b t g", b=B, t=T))

                emit_pass_tiles(0)
                for vcg in range(4):             # groups of 2048 nodes
                    xbs = []
                    for b in range(B):
                        for tq in range(2):
                            xb = xb_pool.tile([128, 2048], bf16, tag="xb",
                                              name=f"xb_{vcg}_{b}_{tq}")
                            nc.sync.dma_start(xb[:], x_d[b, tq, vcg])
                            xbs.append(xb)
                    for vc4 in range(4):
                        for vb4 in range(4):
                            vblk = vcg * 16 + vc4 * 4 + vb4
                            hps = hps_pool.tile([128, BTG], f32, tag="hps",
                                                name=f"hps_{vblk}")
                            for bt in range(4):
                                c0 = vc4 * 512 + vb4 * 128
                                nc.tensor.matmul(
                                    hps[:, bt * 128:(bt + 1) * 128],
                                    lhsT=xbs[bt][:, c0:c0 + 128],
                                    rhs=wblk[:], start=True, stop=True)
                            if with_bias:
                                nc.vector.scalar_tensor_tensor(
                                    hps[:], hps[:], 1.0, wb_sb[:],
                                    op0=OP.bypass, op1=OP.add)
                            hsl = hb_sb[:, vblk * BTG:(vblk + 1) * BTG]
                            nc.scalar.activation(hsl, hps[:], AF.Prelu,
                                                 bias=0.0, scale=1.0,
                                                 alpha=ALPHA)
                            # fused s accumulation: scratch out + row sum
                            sc = hc_pool.tile([128, BTG], bf16, tag="sc",
                                              name=f"sc_{vblk}")
                            nc.vector.scalar_tensor_tensor(
                                sc[:], hsl, 1.0, wa1[:],
                                op0=OP.bypass, op1=OP.mult,
                                accum_out=ss_t[:, vblk:vblk + 1])
                            if vblk < SHARD // 128:
                                sc2 = hc_pool.tile([128, BTG], bf16,
                                                   tag="sc2",
                                                   name=f"sc2_{vblk}")
                                nc.vector.scalar_tensor_tensor(
                                    sc2[:], hsl, 1.0, wa2[:],
                                    op0=OP.bypass, op1=OP.mult,
                                    accum_out=sdcol[:, vblk:vblk + 1])
                    if vcg == 0:
                        # own-shard s_dst complete: broadcast via DRAM
                        nc.gpsimd.dma_start(
                            sd_scr.rearrange("(c p) -> p c", p=128), sdcol[:])
                        sdrep = consts.tile([128, SHARD], f32)
                        nc.gpsimd.dma_start(sdrep[:],
                                            sd_scr.partition_broadcast(128))
                        nc.scalar.activation(grep[:], sdrep[:], AF.Exp,
                                             bias=0.0, scale=-0.8)
                    # per-group exps for s_src columns
                    lo, hi = vcg * 16, (vcg + 1) * 16
                    nc.scalar.activation(acol[:, lo:hi], ss_t[:, lo:hi],
                                         AF.Exp, bias=0.0, scale=0.2)
                    nc.scalar.activation(e2col[:, lo:hi], ss_t[:, lo:hi],
                                         AF.Exp, bias=0.0, scale=1.0)
                    # C-pass-0 for the previous group's j-blocks
                    if vcg >= 1:
                        g = vcg - 1
                        emit_jbg(0, 2 * g, nc.gpsimd)
                        emit_jbg(0, 2 * g + 1, nc.scalar)

                # C-pass-0 tail (last group) + finals
                emit_jbg(0, 6, nc.gpsimd)
                emit_jbg(0, 7, nc.scalar)
                emit_finals(0)
                # C-pass-1
                emit_pass_tiles(1)
                for jbg in range(NVB // JGRP):
                    emit_jbg(1, jbg, nc.sync if jbg % 2 == 0 else nc.scalar)
                emit_finals(1)

    nc.compile()
    return nc


def _host_constants(W_w, W_b, Wt_w, a_w):
    bf = ml_dtypes.bfloat16
    wblk = np.zeros((128, 128), np.float32)
    wT = np.asarray(W_w, np.float32).T           # [f, g]
    for t in range(4):
        wblk[t * 32:(t + 1) * 32, t * 32:(t + 1) * 32] = wT
    # wa[c] for c=(b,tq,t4,g): (Wt_w[t]/B) * a_w[g], replicated to 128 rows
    wt = np.asarray(Wt_w, np.float64) / B
    a1 = np.asarray(a_w[:F1], np.float64)
    a2 = np.asarray(a_w[F1:], np.float64)
    c_t = np.tile(np.repeat(wt, F1), B)
    wa1 = (c_t * np.tile(a1, B * T)).astype(np.float32)
    wa2 = (c_t * np.tile(a2, B * T)).astype(np.float32)
    wa1_rep = np.broadcast_to(wa1.astype(bf), (128, BTG)).copy()
    wa2_rep = np.broadcast_to(wa2.astype(bf), (128, BTG)).copy()
    return wblk.astype(bf), wa1_rep, wa2_rep


def _make_in_maps(x, W_w, W_b, Wt_w, a_w, adj, with_bias):
    wblk, wa1_rep, wa2_rep = _host_constants(W_w, W_b, Wt_w, a_w)
    bf = ml_dtypes.bfloat16
    xt_base = np.ascontiguousarray(
        x.reshape(B, V, 2, 128).transpose(0, 2, 3, 1)).astype(bf)
    adj_t = np.ascontiguousarray(adj.T)          # [j, i_global]

    in_maps = []
    for c in range(NCORES):
        s = c * SHARD
        xt_roll = np.roll(xt_base, -s, axis=3)
        xt = np.ascontiguousarray(
            xt_roll.reshape(B, 2, 128, 4, 2048).transpose(0, 1, 3, 2, 4))
        at = adj_t[:, s:s + SHARD]
        adj_ts = np.concatenate([at[s:, :], at[:s, :]], axis=0)
        adjt = np.ascontiguousarray(
            adj_ts.reshape(NVB // JGRP, JGRP, 128, NPASS, IHALF)
            .transpose(3, 0, 1, 2, 4)).astype(bf)
        m = {"xt": xt, "adjt": adjt, "wblk": wblk, "wa1": wa1_rep,
             "wa2": wa2_rep}
        if with_bias:
            wb_rep = np.tile(np.asarray(W_b, np.float32), B * T)[None, :]
            m["wb"] = np.ascontiguousarray(wb_rep)
        in_maps.append(m)
    return in_maps


def kernel(x, W_w, W_b, Wt_w, a_w, adj):
    from concourse.bass_utils import run_bass_kernel_spmd

    x = np.ascontiguousarray(np.asarray(x, np.float32))
    adj = np.ascontiguousarray(np.asarray(adj, np.int32))
    with_bias = bool(np.any(np.asarray(W_b) != 0))
    if with_bias not in _prog_cache:
        _prog_cache[with_bias] = _build_program(with_bias)
    nc = _prog_cache[with_bias]

    in_maps = _make_in_maps(x, W_w, W_b, Wt_w, a_w, adj, with_bias)
    trace = os.environ.get("KERNEL_TRACE", "0") == "1"
    res = run_bass_kernel_spmd(nc, in_maps, core_ids=list(range(NCORES)),
                               trace=trace)
    kernel.last_results = res
    out = np.concatenate([np.asarray(r["out"], dtype=np.float32)
                          for r in res.results], axis=1)
    return out


kernel.last_results = None
